# revision 93
# baseline (speedup 1.0000x reference)
"""Trainium2 Bass kernel for GNN NodeProcessor (segment_sum + MLP + LayerNorm + residual).

Strategy (8 NeuronCores, SPMD, no collectives):
  - Host: error-feedback-quantize edge_attr to fp8e4m3 along each destination
    node's edge list (carry compensation makes the per-node SUM accurate to
    ~one quantization step instead of sqrt(k) steps). Bucket edges by
    destination node-tile (TW nodes per tile), assign tiles to cores
    rank-matched by block count so one static schedule covers all 8 cores.
    All device tensors are laid out partition-major on host so every DMA line
    is a single long contiguous descriptor (KBs) instead of 512B packets.
  - Device phase A (aggregation): for each node tile, accumulate
        aggT[f, n] += edge_block[e, f].T @ onehot[e, n]
    in PSUM. onehot is built on device (one DVE is_equal per S-block group
    against a tiled iota row, j indices broadcast via stride-0 middle dim)
    from fp16 local destination indices - no onehot DMA traffic.
  - Device phase B (MLP + LN): in [feat, node] layout,
        h1 = silu(W1.T-chunks @ [xT; aggT] + b1), h2 = W2-chunks @ h1 + b2,
    transpose back to [node, feat] with TensorE (fp16), batched bn_stats
    reading the transpose PSUM directly, rstd via Quake-rsqrt + Newton on
    DVE, normalize on ScalarE (scale=rstd, bias=-mu*rstd), residual from a
    partition-major fp16 x copy on DVE, fp16 output.
  - Host: reassemble node tiles into the full [N, D] f32 output.
"""

import numpy as np

P = 128
D = 128
N_CORES = 8
TW = 32              # aggregation node-tile width
GROUP = 512          # nodes per MLP group (max matmul free dim fp32 PSUM)

# wc (fp16 packed consts) column offsets
WC_W1A = 0        # W1[0:128, :]   (256 cols)
WC_W1B = 256      # W1[128:256, :] (256 cols)
WC_W2A = 512      # W2[0:128, :]   (128 cols)
WC_W2B = 640      # W2[128:256, :] (128 cols)
WC_IOTAR = 768    # tiled iota: col s*TW+n = n (TW*S_OH cols)
S_OH = 16         # onehot blocks per DVE build (fixed for wc layout)
WC_COLS = 768 + TW * S_OH


class Cfg:
    def __init__(self, n_nodes=50000, tpc=50, dt_edge="float8e4", eb=64,
                 eps=1e-5, native_silu=True, at=16, newton=1, oh_frac=0.85):
        # the last oh_frac of onehot supers are DMA'd from host (fp8); the
        # early ones are built on DVE (which is idle early, busy late with
        # LN work). oh_frac=0 disables the DMA path.
        self.oh_frac = oh_frac
        self.n_nodes = n_nodes
        self.tpc = tpc                    # 128-node tiles per core
        self.dt_edge = dt_edge
        self.eb = eb                      # edge blocks per DMA batch
        self.eps = eps
        self.native_silu = native_silu
        self.at = at                      # agg tiles per PSUM accumulator
        self.newton = newton              # Newton iterations for rsqrt
        self.apc = tpc * P // TW          # aggregation tiles per core
        self.nta = N_CORES * self.apc     # global aggregation tiles
        self.npc = tpc * P                # padded nodes per core


REAL_CFG = Cfg()

LAST_RESULTS = None  # BassKernelResults of the most recent run (for test.py)


# ---------------------------------------------------------------- host prep

def _ef_quantize(ea, j, n_nodes, np_q):
    """Quantize ea to np_q with error feedback along each node's edge list:
    the carry makes each node's per-feature SUM of quantized values track the
    exact sum to ~one quantization step."""
    order = np.argsort(j, kind="stable")
    cnt = np.bincount(j, minlength=n_nodes)
    starts = np.concatenate([[0], np.cumsum(cnt)[:-1]])
    ea_q = np.zeros(ea.shape, dtype=np_q)
    carry = np.zeros((n_nodes, ea.shape[1]), dtype=np.float32)
    for i in range(int(cnt.max())):
        active = cnt > i
        idx = order[starts[active] + i]
        v = ea[idx] + carry[active]
        q = v.astype(np_q)
        carry[active] = v - q.astype(np.float32)
        ea_q[idx] = q
    return ea_q


def _prepare(x, edge_index, edge_attr, beta, cfg):
    import concourse.mybir as mybir

    np_edge = mybir.dt.np(getattr(mybir.dt, cfg.dt_edge))
    j = np.asarray(edge_index)[1].astype(np.int64)
    ea = np.asarray(edge_attr, dtype=np.float32)
    x = np.asarray(x, dtype=np.float32)

    if cfg.dt_edge.startswith("float8"):
        ea_q = _ef_quantize(ea, j, cfg.n_nodes, np_edge)
    else:
        ea_q = ea.astype(np_edge)

    gtile = j // TW
    cnt = np.bincount(gtile, minlength=cfg.nta)
    bpt = np.maximum((cnt + P - 1) // P, 1)  # edge blocks per tile (>=1)

    order = np.argsort(-bpt, kind="stable")
    gmap = order[: cfg.nta].reshape(cfg.apc, N_CORES).T  # [core, pos] tile id
    schedule = [int(bpt[order[k * N_CORES]]) for k in range(cfg.apc)]
    nb = sum(schedule)
    base = np.concatenate([[0], np.cumsum(schedule)])

    eorder = np.argsort(gtile, kind="stable")
    tstart = np.concatenate([[0], np.cumsum(cnt)])

    shards = []
    node_ids_all = []
    for c in range(N_CORES):
        attr_rows = np.zeros((nb * P, D), dtype=np_edge)
        jl = np.zeros((nb * P,), dtype=np.float16)
        for k in range(cfg.apc):
            gt = int(gmap[c, k])
            s0 = int(tstart[gt])
            csz = int(cnt[gt])
            if csz == 0:
                continue
            rows = eorder[s0 : s0 + csz]
            dst0 = int(base[k]) * P
            attr_rows[dst0 : dst0 + csz] = ea_q[rows]
            jl[dst0 : dst0 + csz] = (j[rows] - gt * TW).astype(np.float16)
        edges_pm = np.ascontiguousarray(
            attr_rows.reshape(nb, P, D).transpose(1, 0, 2).reshape(P, nb * D)
        )
        j_pm = np.ascontiguousarray(jl.reshape(nb, P).T)  # [P, nb]

        # fp8 onehot rows for the DMA'd (late) supers, compact layout
        nsup = (nb + S_OH - 1) // S_OH
        split = nsup - int(nsup * cfg.oh_frac)
        dma_sups = list(range(split, nsup))
        oh_rows = np.zeros((len(dma_sups) * S_OH * P, TW), dtype=np_edge)
        jli = jl.astype(np.int64)
        for di, s in enumerate(dma_sups):
            b0, b1 = s * S_OH, min((s + 1) * S_OH, nb)
            src = slice(b0 * P, b1 * P)
            dst0 = di * S_OH * P
            n_e = (b1 - b0) * P
            rows_idx = np.arange(dst0, dst0 + n_e)
            oh_rows[rows_idx, jli[src]] = 1.0
            # padding edges have jl==0 and zero features: the stray 1.0 in
            # col 0 multiplies zero features, contributing nothing
        nds = len(dma_sups)
        oh_pm = np.ascontiguousarray(
            oh_rows.reshape(nds * S_OH, P, TW)
            .transpose(1, 0, 2)
            .reshape(P, nds * S_OH * TW)
        ) if nds else np.zeros((P, S_OH * TW), dtype=np_edge)

        node_ids = (gmap[c][:, None] * TW + np.arange(TW)[None, :]).reshape(-1)
        valid = node_ids < cfg.n_nodes
        xs = x[np.minimum(node_ids, cfg.n_nodes - 1)] * valid[:, None]
        xt = np.ascontiguousarray(xs.T.astype(np.float16))  # [D, NPC]
        xsb = (xs + beta[None, :]).astype(np.float16)
        xs_pm = np.ascontiguousarray(
            xsb.reshape(cfg.tpc, P, D).transpose(1, 0, 2).reshape(P, cfg.tpc * D)
        )
        shards.append({"edges": edges_pm, "jt": j_pm, "oh": oh_pm,
                       "xs": xs_pm, "xt": xt})
        node_ids_all.append(node_ids)

    return schedule, shards, node_ids_all


# ---------------------------------------------------------------- device build

def _build(cfg, schedule, gamma_one):
    import concourse.bacc as bacc
    import concourse.mybir as mybir
    import concourse.tile as tile
    from concourse.masks import make_identity

    dt = mybir.dt
    f32 = dt.float32
    f16 = dt.float16
    u32 = dt.uint32
    dte = getattr(dt, cfg.dt_edge)
    Af = mybir.ActivationFunctionType
    Op = mybir.AluOpType

    NPC = cfg.npc
    NB = sum(schedule)
    TPC = cfg.tpc
    EB = cfg.eb
    NEB = (NB + EB - 1) // EB

    NSUP = (NB + S_OH - 1) // S_OH
    SPLIT = NSUP - int(NSUP * cfg.oh_frac)
    NDS = max(NSUP - SPLIT, 1)
    OBS = 8  # DMA'd onehot supers per fetch

    nc = bacc.Bacc(None)
    edges_d = nc.dram_tensor("edges", [P, NB * D], dte, kind="ExternalInput")
    oh_d = nc.dram_tensor("oh", [P, NDS * S_OH * TW], dte, kind="ExternalInput")
    xs_d = nc.dram_tensor("xs", [P, TPC * D], f16, kind="ExternalInput")
    xt_d = nc.dram_tensor("xt", [D, NPC], f16, kind="ExternalInput")
    wc_d = nc.dram_tensor("wc", [P, WC_COLS + NB], f16, kind="ExternalInput")
    fc_d = nc.dram_tensor("fc", [P, 4], f32, kind="ExternalInput")
    gam_d = nc.dram_tensor("gam", [P, D], f32, kind="ExternalInput")
    out_d = nc.dram_tensor("out", [P, TPC * D], f16, kind="ExternalOutput")

    with tile.TileContext(nc) as tc:
        with (
            tc.tile_pool(name="const", bufs=1) as cpool,
            tc.tile_pool(name="ebatch", bufs=5) as epool,
            tc.tile_pool(name="oh", bufs=4) as ohpool,
            tc.tile_pool(name="ohd", bufs=3) as ohdpool,
            tc.tile_pool(name="agg", bufs=3) as aggpool,
            tc.tile_pool(name="h1s", bufs=3) as h1pool,
            tc.tile_pool(name="h2s", bufs=3) as h2pool,
            tc.tile_pool(name="ln", bufs=8) as lnpool,
            tc.tile_pool(name="stat", bufs=16) as stpool,
            tc.tile_pool(name="ost", bufs=4) as ospool,
            tc.tile_pool(name="psA", bufs=2, space="PSUM") as psA,
            tc.tile_pool(name="psB", bufs=1, space="PSUM") as psB,
            tc.tile_pool(name="psT", bufs=2, space="PSUM") as psT,
        ):
            # ---- constants / full loads (small first, edges prefetched
            # below). wc carries the packed weights, iota AND j indices in
            # one DMA (fewer serial descriptor-gens on the Sync engine).
            wcj_sb = cpool.tile([P, WC_COLS + NB], f16, tag="wc")
            nc.sync.dma_start(out=wcj_sb[:], in_=wc_d[:, :])
            wc_sb = wcj_sb
            jt_sb = wcj_sb[:, WC_COLS : WC_COLS + NB]
            fc_sb = cpool.tile([P, 4], f32, tag="fc")
            nc.sync.dma_start(out=fc_sb[:], in_=fc_d[:, :])

            ebatches = {}

            def fetch_batch(bi):
                if bi in ebatches or bi >= NEB:
                    return
                nbe = min(EB, NB - bi * EB)
                t = epool.tile([P, EB * D], dte, tag="eb")
                nc.sync.dma_start(
                    out=t[:, : nbe * D],
                    in_=edges_d[:, bi * EB * D : bi * EB * D + nbe * D],
                )
                ebatches[bi] = t

            obatches = {}

            def fetch_ohd(obi):
                # DMA'd onehot supers ride the GpSimd queue
                if obi in obatches or obi * OBS >= NSUP - SPLIT:
                    return
                nsb = min(OBS, NSUP - SPLIT - obi * OBS)
                w = S_OH * TW
                t = ohdpool.tile([P, OBS * w], dte, tag="ohd")
                nc.scalar.dma_start(
                    out=t[:, : nsb * w],
                    in_=oh_d[:, obi * OBS * w : (obi * OBS + nsb) * w],
                )
                obatches[obi] = t

            for _bi in range(4):
                fetch_batch(_bi)

            # x loads ride the Sync queue BEHIND the first edge batches:
            # FIFO order keeps the PE-critical early edge stream ahead of
            # them, and they still land before the first phase-B needs them
            xt_sb = cpool.tile([D, NPC], f16, tag="xt")
            nc.sync.dma_start(out=xt_sb[:], in_=xt_d[:, :])
            xs_sb = cpool.tile([P, TPC * D], f16, tag="xs")
            nc.sync.dma_start(out=xs_sb[:], in_=xs_d[:, :])
            if not gamma_one:
                gam_sb = cpool.tile([P, D], f32, tag="gam")
                nc.gpsimd.dma_start(out=gam_sb[:], in_=gam_d[:, :])

            ident = cpool.tile([P, P], f32, tag="ident")
            make_identity(nc, ident[:])

            # Quake rsqrt integer constants (per-partition scalars)
            c_shift = cpool.tile([P, 1], u32, tag="cshift")
            nc.vector.memset(c_shift[:], 1)
            c_magic = cpool.tile([P, 1], u32, tag="cmagic")
            nc.vector.memset(c_magic[:], 0x5F3759DF)
            c_m05 = cpool.tile([P, 1], f32, tag="cm05")
            nc.vector.memset(c_m05[:], -0.5)
            c_15 = cpool.tile([P, 1], f32, tag="c15")
            nc.vector.memset(c_15[:], 1.5)

            # ---- group structure over this core's nodes; small tail groups
            # shrink the serial pipeline-drain at the end
            groups = []
            n0 = 0
            tails = [256, 256, 256] if NPC >= 2 * GROUP else []
            tail = sum(tails)
            while n0 < NPC - tail:
                gw = min(GROUP, NPC - tail - n0)
                groups.append((n0, gw))
                n0 += gw
            for gw in tails:
                groups.append((n0, gw))
                n0 += gw

            state = {"blk": 0, "cur_sup": -1, "oh_t": None}

            def do_agg(n0, gw):
                # ---- phase A: aggregate this group's node tiles
                agg_g = aggpool.tile([P, GROUP], f16, tag="agg")
                ntile = gw // TW
                a = 0
                while a < ntile:
                    atc = min(cfg.at, ntile - a)
                    ps = psA.tile([P, cfg.at * TW], f32, tag="psA")
                    for a2 in range(atc):
                        k = n0 // TW + a + a2
                        nblk = schedule[k]
                        for i in range(nblk):
                            blk = state["blk"]
                            bi = blk // EB
                            if bi not in ebatches:
                                fetch_batch(bi)
                            fetch_batch(bi + 1)
                            ebatches.pop(bi - 2, None)
                            sup = blk // S_OH
                            so = blk - sup * S_OH
                            lookahead = (blk + 8 * S_OH) // S_OH
                            if lookahead >= SPLIT:
                                fetch_ohd((lookahead - SPLIT) // OBS)
                            if sup >= SPLIT:
                                # DMA'd onehot super (fp8)
                                di = sup - SPLIT
                                obi = di // OBS
                                if obi not in obatches:
                                    fetch_ohd(obi)
                                fetch_ohd(obi + 1)
                                obatches.pop(obi - 2, None)
                                c0 = ((di - obi * OBS) * S_OH + so) * TW
                                rhs = obatches[obi][:, c0 : c0 + TW]
                            else:
                                if sup != state["cur_sup"]:
                                    # onehot[p, s*TW+n] = (j[.] == n);
                                    # contiguous per block for matmul rhs
                                    nbk = min(S_OH, NB - sup * S_OH)
                                    oh_t = ohpool.tile(
                                        [P, TW * S_OH], f16, tag="oh"
                                    )
                                    nc.vector.tensor_tensor(
                                        out=oh_t[:, : nbk * TW].rearrange(
                                            "p (s n) -> p s n", n=TW
                                        ),
                                        in0=wc_sb[
                                            :, WC_IOTAR : WC_IOTAR + nbk * TW
                                        ].rearrange("p (s n) -> p s n", n=TW),
                                        in1=jt_sb[
                                            :, sup * S_OH : sup * S_OH + nbk
                                        ]
                                        .unsqueeze(2)
                                        .broadcast_to([P, nbk, TW]),
                                        op=Op.is_equal,
                                    )
                                    state["cur_sup"] = sup
                                    state["oh_t"] = oh_t
                                rhs = state["oh_t"][:, so * TW
                                                    : (so + 1) * TW]
                            nc.tensor.matmul(
                                out=ps[:, a2 * TW : (a2 + 1) * TW],
                                lhsT=ebatches[bi][
                                    :, (blk - bi * EB) * D
                                    : (blk - bi * EB + 1) * D
                                ],
                                rhs=rhs,
                                start=(i == 0),
                                stop=(i == nblk - 1),
                            )
                            state["blk"] = blk + 1
                    nc.scalar.copy(
                        agg_g[:, a * TW : (a + atc) * TW], ps[:, : atc * TW]
                    )
                    a += atc
                return {"n0": n0, "gw": gw, "agg_g": agg_g}

            def stage1(st):
                # ---- MLP for this group (through h2 PSUM)
                n0, gw, agg_g = st["n0"], st["gw"], st["agg_g"]
                nsl = slice(n0, n0 + gw)
                h1a_ps = psB.tile([P, GROUP], f32, tag="h1a")
                h1b_ps = psB.tile([P, GROUP], f32, tag="h1b")
                nc.tensor.matmul(
                    out=h1a_ps[:, :gw], lhsT=wc_sb[:, WC_W1A : WC_W1A + P],
                    rhs=xt_sb[:, nsl], start=True, stop=False,
                )
                nc.tensor.matmul(
                    out=h1a_ps[:, :gw], lhsT=wc_sb[:, WC_W1B : WC_W1B + P],
                    rhs=agg_g[:, :gw], start=False, stop=True,
                )
                nc.tensor.matmul(
                    out=h1b_ps[:, :gw], lhsT=wc_sb[:, WC_W1A + P : WC_W1A + 2 * P],
                    rhs=xt_sb[:, nsl], start=True, stop=False,
                )
                nc.tensor.matmul(
                    out=h1b_ps[:, :gw], lhsT=wc_sb[:, WC_W1B + P : WC_W1B + 2 * P],
                    rhs=agg_g[:, :gw], start=False, stop=True,
                )
                h1a_sb = h1pool.tile([P, GROUP], f16, tag="h1as")
                h1b_sb = h1pool.tile([P, GROUP], f16, tag="h1bs")
                if cfg.native_silu:
                    nc.scalar.activation(
                        h1a_sb[:, :gw], h1a_ps[:, :gw], Af.Silu,
                        bias=fc_sb[:, 0:1],
                    )
                    nc.scalar.activation(
                        h1b_sb[:, :gw], h1b_ps[:, :gw], Af.Silu,
                        bias=fc_sb[:, 1:2],
                    )
                else:
                    # silu(z) = z * sigmoid(z), z = h1 + b1 (CoreSim lacks Silu)
                    for (hps, hsb, bsl) in (
                        (h1a_ps, h1a_sb, fc_sb[:, 0:1]),
                        (h1b_ps, h1b_sb, fc_sb[:, 1:2]),
                    ):
                        zpre = h1pool.tile([P, GROUP], f16, tag="zpre")
                        nc.scalar.activation(
                            zpre[:, :gw], hps[:, :gw], Af.Identity, bias=bsl
                        )
                        zsig = h1pool.tile([P, GROUP], f16, tag="zsig")
                        nc.scalar.activation(
                            zsig[:, :gw], hps[:, :gw], Af.Sigmoid, bias=bsl
                        )
                        nc.vector.tensor_tensor(
                            out=hsb[:, :gw], in0=zpre[:, :gw], in1=zsig[:, :gw],
                            op=Op.mult,
                        )
                h2_ps = psB.tile([P, GROUP], f32, tag="h2", bufs=2)
                nc.tensor.matmul(
                    out=h2_ps[:, :gw], lhsT=wc_sb[:, WC_W2A : WC_W2A + P],
                    rhs=h1a_sb[:, :gw], start=True, stop=False,
                )
                nc.tensor.matmul(
                    out=h2_ps[:, :gw], lhsT=wc_sb[:, WC_W2B : WC_W2B + P],
                    rhs=h1b_sb[:, :gw], start=False, stop=True,
                )
                st["h2_ps"] = h2_ps

            def stage2a(st):
                gw = st["gw"]
                h2_sb = h2pool.tile([P, GROUP], f32, tag="h2s")
                nc.scalar.activation(
                    h2_sb[:, :gw], st["h2_ps"][:, :gw], Af.Identity,
                    bias=fc_sb[:, 2:3],
                )
                st["h2_sb"] = h2_sb

            def stage2b(st):
                n0, gw, h2_sb = st["n0"], st["gw"], st["h2_sb"]
                # ---- transpose to [node, feat], tile pairs interleaved
                # column-wise in PSUM so one bn_stats over the contiguous
                # [P, 256] pair-region yields per-tile stats in its even/odd
                # halves (count, mean, count*var each) - no bn_aggr needed.
                nq = gw // P
                nh = (nq + 1) // 2
                tp_all = psT.tile([P, 4 * P], f32, tag="tp")
                pairviews = []
                for h in range(nh):
                    region = tp_all[:, h * 2 * P : (h + 1) * 2 * P]
                    pv = region.rearrange("p (f two) -> p f two", two=2)
                    pairviews.append(pv)
                    for t in range(2):
                        q = min(2 * h + t, nq - 1)  # odd tail: dup last tile
                        nc.tensor.transpose(
                            pv[:, :, t], h2_sb[:, q * P : (q + 1) * P],
                            ident[:],
                        )
                stats_g = stpool.tile([P, 12], f32, tag="bst")
                for h in range(nh):
                    nc.vector.bn_stats(
                        out=stats_g[:, 6 * h : 6 * h + 6],
                        in_=tp_all[:, h * 2 * P : (h + 1) * 2 * P],
                    )

                # ---- rstd = 1/sqrt(var + eps) via Quake rsqrt + Newton
                # stats cols: [1]=mean_even(q0) [2]=128*var(q0)
                #             [4]=mean_odd(q1)  [5]=128*var(q1)
                veps = stpool.tile([P, 4], f32, tag="veps")
                nc.vector.tensor_scalar(
                    out=veps[:, :nq],
                    in0=stats_g[:, 2 : 3 * nq : 3],
                    scalar1=1.0 / P, scalar2=cfg.eps,
                    op0=Op.mult, op1=Op.add,
                )
                ish = stpool.tile([P, 4], u32, tag="ish")
                nc.vector.tensor_tensor(
                    out=ish[:, :nq], in0=veps[:, :nq].bitcast(u32),
                    in1=c_shift[:, :].to_broadcast([P, nq]),
                    op=Op.arith_shift_right,
                )
                y = stpool.tile([P, 4], f32, tag="rsy")
                nc.vector.tensor_tensor(
                    out=y[:, :nq].bitcast(u32),
                    in0=c_magic[:, :].to_broadcast([P, nq]),
                    in1=ish[:, :nq], op=Op.subtract,
                )
                for it in range(cfg.newton):
                    t_ = stpool.tile([P, 4], f32, tag=f"nt{it}")
                    nc.vector.tensor_tensor(
                        out=t_[:, :nq], in0=y[:, :nq], in1=y[:, :nq], op=Op.mult
                    )
                    nc.vector.tensor_tensor(
                        out=t_[:, :nq], in0=t_[:, :nq], in1=veps[:, :nq], op=Op.mult
                    )
                    nc.vector.tensor_scalar(
                        out=t_[:, :nq], in0=t_[:, :nq], scalar1=c_m05[:, :],
                        scalar2=c_15[:, :], op0=Op.mult, op1=Op.add,
                    )
                    y2 = stpool.tile([P, 4], f32, tag=f"ny{it}")
                    nc.vector.tensor_tensor(
                        out=y2[:, :nq], in0=y[:, :nq], in1=t_[:, :nq], op=Op.mult
                    )
                    y = y2

                # ---- normalize on ScalarE: (tp - mu)*rstd = tp*rstd + (-mu*rstd)
                nmu = stpool.tile([P, 4], f32, tag="nmu")
                nc.vector.scalar_tensor_tensor(
                    out=nmu[:, :nq], in0=stats_g[:, 1 : 3 * nq : 3],
                    scalar=-1.0, in1=y[:, :nq],
                    op0=Op.mult, op1=Op.mult,
                )
                ost = ospool.tile([P, GROUP], f16, tag="ost")
                for q in range(nq):
                    k = n0 // P + q
                    t1 = lnpool.tile([P, P], f16, tag="t1")
                    if q % 2 == 0:
                        nc.scalar.activation(
                            t1[:], pairviews[q // 2][:, :, q % 2],
                            Af.Identity,
                            bias=nmu[:, q : q + 1], scale=y[:, q : q + 1],
                        )
                    else:
                        nc.vector.tensor_scalar(
                            out=t1[:], in0=pairviews[q // 2][:, :, q % 2],
                            scalar1=stats_g[:, 1 + 3 * q : 2 + 3 * q],
                            scalar2=y[:, q : q + 1],
                            op0=Op.subtract, op1=Op.mult,
                        )
                    if gamma_one:
                        t2 = t1
                    else:
                        t2 = lnpool.tile([P, P], f16, tag="t2")
                        nc.vector.tensor_tensor(
                            out=t2[:], in0=t1[:], in1=gam_sb[:], op=Op.mult
                        )
                    nc.vector.tensor_tensor(
                        out=ost[:, q * P : (q + 1) * P], in0=t2[:],
                        in1=xs_sb[:, k * D : (k + 1) * D], op=Op.add,
                    )
                nc.gpsimd.dma_start(
                    out=out_d[:, n0 : n0 + gw], in_=ost[:, :gw]
                )

            # ---- software-pipelined driver: phase-B lags aggregation by
            # one group (MLP) / two groups (LN), so no engine's in-order
            # queue head-of-line blocks on a cross-engine dependency.
            pipe = []
            for (n0, gw) in groups:
                if len(pipe) == 2:
                    stage2a(pipe[0])
                if len(pipe) >= 1:
                    stage1(pipe[-1])
                if len(pipe) == 2:
                    stage2b(pipe.pop(0))
                pipe.append(do_agg(n0, gw))
            if len(pipe) == 2:
                stage2a(pipe[0])
            stage1(pipe[-1])
            if len(pipe) == 2:
                stage2b(pipe.pop(0))
            stage2a(pipe[0])
            stage2b(pipe.pop(0))
    nc.finalize()
    return nc


# ---------------------------------------------------------------- run

def _run(inputs, cfg, use_sim=False):
    global LAST_RESULTS
    x = np.asarray(inputs["x"], dtype=np.float32)
    W1 = np.asarray(inputs["W1"], dtype=np.float16)
    W2 = np.asarray(inputs["W2"], dtype=np.float16)
    b1 = np.asarray(inputs["b1"], dtype=np.float32)
    b2 = np.asarray(inputs["b2"], dtype=np.float32)
    beta = np.asarray(inputs["ln_beta"], dtype=np.float32)
    gamma = np.asarray(inputs["ln_gamma"], dtype=np.float32)
    gamma_one = bool(np.all(gamma == 1.0))

    schedule, shards, node_ids = _prepare(
        x, inputs["edge_index"], inputs["edge_attr"], beta, cfg
    )
    nc = _build(cfg, schedule, gamma_one)

    wc = np.zeros((P, WC_COLS), dtype=np.float16)
    wc[:, WC_W1A : WC_W1A + 2 * D] = W1[0:P, :]
    wc[:, WC_W1B : WC_W1B + 2 * D] = W1[P : 2 * P, :]
    wc[:, WC_W2A : WC_W2A + D] = W2[0:P, :]
    wc[:, WC_W2B : WC_W2B + D] = W2[P : 2 * P, :]
    iota_rep = np.tile(np.arange(TW, dtype=np.float16), S_OH)
    wc[:, WC_IOTAR : WC_IOTAR + TW * S_OH] = iota_rep[None, :]
    fc = np.zeros((P, 4), dtype=np.float32)
    fc[:, 0] = b1[0:P]
    fc[:, 1] = b1[P : 2 * P]
    fc[:, 2] = b2[0:P]
    gam = np.tile(gamma[None, :], (P, 1)).astype(np.float32)

    common = {"fc": fc, "gam": gam}
    in_maps = []
    for sh in shards:
        m = dict(sh, **common)
        m["wc"] = np.concatenate([wc, m.pop("jt")], axis=1)
        in_maps.append(m)

    if use_sim:
        from concourse import bass_interp

        outs = []
        for c in range(N_CORES):
            sim = bass_interp.MultiCoreSim(nc, 1)
            for name, arr in in_maps[c].items():
                sim.cores[0].tensor(name)[:] = arr
            sim.simulate()
            outs.append(np.array(sim.cores[0].mem_tensor("out")))
    else:
        from concourse.bass_utils import run_bass_kernel_spmd

        res = run_bass_kernel_spmd(nc, in_maps, list(range(N_CORES)))
        LAST_RESULTS = res
        outs = [res.results[c]["out"] for c in range(N_CORES)]

    out_full = np.zeros((cfg.nta * TW, D), dtype=np.float32)
    for c in range(N_CORES):
        out_pm = np.asarray(outs[c])
        out_rows = (
            out_pm.reshape(P, cfg.tpc, D).transpose(1, 0, 2).reshape(cfg.npc, D)
        )
        out_full[node_ids[c]] = out_rows.astype(np.float32)
    return out_full[: cfg.n_nodes]


def kernel(**inputs):
    return _run(inputs, REAL_CFG, use_sim=False)


# revision 94
# speedup vs baseline: 1.0271x; 1.0271x over previous
"""Trainium2 Bass kernel for GNN NodeProcessor (segment_sum + MLP + LayerNorm + residual).

Strategy (8 NeuronCores, SPMD, no collectives):
  - Host: error-feedback-quantize edge_attr to fp8e4m3 along each destination
    node's edge list (carry compensation makes the per-node SUM accurate to
    ~one quantization step instead of sqrt(k) steps). Bucket edges by
    destination node-tile (TW nodes per tile), assign tiles to cores
    rank-matched by block count so one static schedule covers all 8 cores.
    All device tensors are laid out partition-major on host so every DMA line
    is a single long contiguous descriptor (KBs) instead of 512B packets.
  - Device phase A (aggregation): for each node tile, accumulate
        aggT[f, n] += edge_block[e, f].T @ onehot[e, n]
    in PSUM. onehot is built on device (one DVE is_equal per S-block group
    against a tiled iota row, j indices broadcast via stride-0 middle dim)
    from fp16 local destination indices - no onehot DMA traffic.
  - Device phase B (MLP + LN): in [feat, node] layout,
        h1 = silu(W1.T-chunks @ [xT; aggT] + b1), h2 = W2-chunks @ h1 + b2,
    transpose back to [node, feat] with TensorE (fp16), batched bn_stats
    reading the transpose PSUM directly, rstd via Quake-rsqrt + Newton on
    DVE, normalize on ScalarE (scale=rstd, bias=-mu*rstd), residual from a
    partition-major fp16 x copy on DVE, fp16 output.
  - Host: reassemble node tiles into the full [N, D] f32 output.
"""

import numpy as np

P = 128
D = 128
N_CORES = 8
TW = 32              # aggregation node-tile width
GROUP = 512          # nodes per MLP group (max matmul free dim fp32 PSUM)

# wc (fp16 packed consts) column offsets
WC_W1A = 0        # W1[0:128, :]   (256 cols)
WC_W1B = 256      # W1[128:256, :] (256 cols)
WC_W2A = 512      # W2[0:128, :]   (128 cols)
WC_W2B = 640      # W2[128:256, :] (128 cols)
WC_IOTAR = 768    # tiled iota: col s*TW+n = n (TW*S_OH cols)
S_OH = 16         # onehot blocks per DVE build (fixed for wc layout)
WC_COLS = 768 + TW * S_OH


class Cfg:
    def __init__(self, n_nodes=50000, tpc=50, dt_edge="float8e4", eb=64,
                 eps=1e-5, native_silu=True, at=16, newton=1, oh_frac=0.75):
        # the last oh_frac of onehot supers are DMA'd from host (fp8); the
        # early ones are built on DVE (which is idle early, busy late with
        # LN work). oh_frac=0 disables the DMA path.
        self.oh_frac = oh_frac
        self.n_nodes = n_nodes
        self.tpc = tpc                    # 128-node tiles per core
        self.dt_edge = dt_edge
        self.eb = eb                      # edge blocks per DMA batch
        self.eps = eps
        self.native_silu = native_silu
        self.at = at                      # agg tiles per PSUM accumulator
        self.newton = newton              # Newton iterations for rsqrt
        self.apc = tpc * P // TW          # aggregation tiles per core
        self.nta = N_CORES * self.apc     # global aggregation tiles
        self.npc = tpc * P                # padded nodes per core


REAL_CFG = Cfg()

LAST_RESULTS = None  # BassKernelResults of the most recent run (for test.py)


# ---------------------------------------------------------------- host prep

def _ef_quantize(ea, j, n_nodes, np_q):
    """Quantize ea to np_q with error feedback along each node's edge list:
    the carry makes each node's per-feature SUM of quantized values track the
    exact sum to ~one quantization step."""
    order = np.argsort(j, kind="stable")
    cnt = np.bincount(j, minlength=n_nodes)
    starts = np.concatenate([[0], np.cumsum(cnt)[:-1]])
    ea_q = np.zeros(ea.shape, dtype=np_q)
    carry = np.zeros((n_nodes, ea.shape[1]), dtype=np.float32)
    for i in range(int(cnt.max())):
        active = cnt > i
        idx = order[starts[active] + i]
        v = ea[idx] + carry[active]
        q = v.astype(np_q)
        carry[active] = v - q.astype(np.float32)
        ea_q[idx] = q
    return ea_q


def _prepare(x, edge_index, edge_attr, beta, cfg):
    import concourse.mybir as mybir

    np_edge = mybir.dt.np(getattr(mybir.dt, cfg.dt_edge))
    j = np.asarray(edge_index)[1].astype(np.int64)
    ea = np.asarray(edge_attr, dtype=np.float32)
    x = np.asarray(x, dtype=np.float32)

    if cfg.dt_edge.startswith("float8"):
        ea_q = _ef_quantize(ea, j, cfg.n_nodes, np_edge)
    else:
        ea_q = ea.astype(np_edge)

    gtile = j // TW
    cnt = np.bincount(gtile, minlength=cfg.nta)
    bpt = np.maximum((cnt + P - 1) // P, 1)  # edge blocks per tile (>=1)

    order = np.argsort(-bpt, kind="stable")
    gmap = order[: cfg.nta].reshape(cfg.apc, N_CORES).T  # [core, pos] tile id
    schedule = [int(bpt[order[k * N_CORES]]) for k in range(cfg.apc)]
    nb = sum(schedule)
    base = np.concatenate([[0], np.cumsum(schedule)])

    eorder = np.argsort(gtile, kind="stable")
    tstart = np.concatenate([[0], np.cumsum(cnt)])

    shards = []
    node_ids_all = []
    for c in range(N_CORES):
        attr_rows = np.zeros((nb * P, D), dtype=np_edge)
        jl = np.zeros((nb * P,), dtype=np.float16)
        for k in range(cfg.apc):
            gt = int(gmap[c, k])
            s0 = int(tstart[gt])
            csz = int(cnt[gt])
            if csz == 0:
                continue
            rows = eorder[s0 : s0 + csz]
            dst0 = int(base[k]) * P
            attr_rows[dst0 : dst0 + csz] = ea_q[rows]
            jl[dst0 : dst0 + csz] = (j[rows] - gt * TW).astype(np.float16)
        edges_pm = np.ascontiguousarray(
            attr_rows.reshape(nb, P, D).transpose(1, 0, 2).reshape(P, nb * D)
        )
        j_pm = np.ascontiguousarray(jl.reshape(nb, P).T)  # [P, nb]

        # fp8 onehot rows for the DMA'd (late) supers, compact layout
        nsup = (nb + S_OH - 1) // S_OH
        split = nsup - int(nsup * cfg.oh_frac)
        dma_sups = list(range(split, nsup))
        oh_rows = np.zeros((len(dma_sups) * S_OH * P, TW), dtype=np_edge)
        jli = jl.astype(np.int64)
        for di, s in enumerate(dma_sups):
            b0, b1 = s * S_OH, min((s + 1) * S_OH, nb)
            src = slice(b0 * P, b1 * P)
            dst0 = di * S_OH * P
            n_e = (b1 - b0) * P
            rows_idx = np.arange(dst0, dst0 + n_e)
            oh_rows[rows_idx, jli[src]] = 1.0
            # padding edges have jl==0 and zero features: the stray 1.0 in
            # col 0 multiplies zero features, contributing nothing
        nds = len(dma_sups)
        oh_pm = np.ascontiguousarray(
            oh_rows.reshape(nds * S_OH, P, TW)
            .transpose(1, 0, 2)
            .reshape(P, nds * S_OH * TW)
        ) if nds else np.zeros((P, S_OH * TW), dtype=np_edge)

        node_ids = (gmap[c][:, None] * TW + np.arange(TW)[None, :]).reshape(-1)
        valid = node_ids < cfg.n_nodes
        xs = x[np.minimum(node_ids, cfg.n_nodes - 1)] * valid[:, None]
        xt = np.ascontiguousarray(xs.T.astype(np.float16))  # [D, NPC]
        xsb = (xs + beta[None, :]).astype(np.float16)
        xs_pm = np.ascontiguousarray(
            xsb.reshape(cfg.tpc, P, D).transpose(1, 0, 2).reshape(P, cfg.tpc * D)
        )
        shards.append({"edges": edges_pm, "jt": j_pm, "oh": oh_pm,
                       "xs": xs_pm, "xt": xt})
        node_ids_all.append(node_ids)

    return schedule, shards, node_ids_all


# ---------------------------------------------------------------- device build

def _build(cfg, schedule, gamma_one):
    import concourse.bacc as bacc
    import concourse.mybir as mybir
    import concourse.tile as tile
    from concourse.masks import make_identity

    dt = mybir.dt
    f32 = dt.float32
    f16 = dt.float16
    u32 = dt.uint32
    dte = getattr(dt, cfg.dt_edge)
    Af = mybir.ActivationFunctionType
    Op = mybir.AluOpType

    NPC = cfg.npc
    NB = sum(schedule)
    TPC = cfg.tpc
    EB = cfg.eb
    NEB = (NB + EB - 1) // EB

    NSUP = (NB + S_OH - 1) // S_OH
    SPLIT = NSUP - int(NSUP * cfg.oh_frac)
    NDS = max(NSUP - SPLIT, 1)
    OBS = 8  # DMA'd onehot supers per fetch

    nc = bacc.Bacc(None)
    edges_d = nc.dram_tensor("edges", [P, NB * D], dte, kind="ExternalInput")
    oh_d = nc.dram_tensor("oh", [P, NDS * S_OH * TW], dte, kind="ExternalInput")
    xs_d = nc.dram_tensor("xs", [P, TPC * D], f16, kind="ExternalInput")
    xt_d = nc.dram_tensor("xt", [D, NPC], f16, kind="ExternalInput")
    wc_d = nc.dram_tensor("wc", [P, WC_COLS + NB], f16, kind="ExternalInput")
    fc_d = nc.dram_tensor("fc", [P, 4], f32, kind="ExternalInput")
    gam_d = nc.dram_tensor("gam", [P, D], f32, kind="ExternalInput")
    out_d = nc.dram_tensor("out", [P, TPC * D], f16, kind="ExternalOutput")

    with tile.TileContext(nc) as tc:
        with (
            tc.tile_pool(name="const", bufs=1) as cpool,
            tc.tile_pool(name="ebatch", bufs=5) as epool,
            tc.tile_pool(name="oh", bufs=4) as ohpool,
            tc.tile_pool(name="ohd", bufs=3) as ohdpool,
            tc.tile_pool(name="agg", bufs=3) as aggpool,
            tc.tile_pool(name="h1s", bufs=3) as h1pool,
            tc.tile_pool(name="h2s", bufs=3) as h2pool,
            tc.tile_pool(name="ln", bufs=8) as lnpool,
            tc.tile_pool(name="stat", bufs=16) as stpool,
            tc.tile_pool(name="ost", bufs=4) as ospool,
            tc.tile_pool(name="psA", bufs=2, space="PSUM") as psA,
            tc.tile_pool(name="psB", bufs=1, space="PSUM") as psB,
            tc.tile_pool(name="psT", bufs=2, space="PSUM") as psT,
        ):
            # ---- constants / full loads (small first, edges prefetched
            # below). wc carries the packed weights, iota AND j indices in
            # one DMA (fewer serial descriptor-gens on the Sync engine).
            wcj_sb = cpool.tile([P, WC_COLS + NB], f16, tag="wc")
            nc.sync.dma_start(out=wcj_sb[:], in_=wc_d[:, :])
            wc_sb = wcj_sb
            jt_sb = wcj_sb[:, WC_COLS : WC_COLS + NB]
            fc_sb = cpool.tile([P, 4], f32, tag="fc")
            nc.sync.dma_start(out=fc_sb[:], in_=fc_d[:, :])

            ebatches = {}

            def fetch_batch(bi):
                if bi in ebatches or bi >= NEB:
                    return
                nbe = min(EB, NB - bi * EB)
                t = epool.tile([P, EB * D], dte, tag="eb")
                nc.sync.dma_start(
                    out=t[:, : nbe * D],
                    in_=edges_d[:, bi * EB * D : bi * EB * D + nbe * D],
                )
                ebatches[bi] = t

            obatches = {}

            def fetch_ohd(obi):
                # DMA'd onehot supers ride the GpSimd queue
                if obi in obatches or obi * OBS >= NSUP - SPLIT:
                    return
                nsb = min(OBS, NSUP - SPLIT - obi * OBS)
                w = S_OH * TW
                t = ohdpool.tile([P, OBS * w], dte, tag="ohd")
                nc.scalar.dma_start(
                    out=t[:, : nsb * w],
                    in_=oh_d[:, obi * OBS * w : (obi * OBS + nsb) * w],
                )
                obatches[obi] = t

            for _bi in range(4):
                fetch_batch(_bi)

            # x loads ride the Sync queue BEHIND the first edge batches:
            # FIFO order keeps the PE-critical early edge stream ahead of
            # them, and they still land before the first phase-B needs them
            xt_sb = cpool.tile([D, NPC], f16, tag="xt")
            nc.sync.dma_start(out=xt_sb[:], in_=xt_d[:, :])
            xs_sb = cpool.tile([P, TPC * D], f16, tag="xs")
            nc.sync.dma_start(out=xs_sb[:], in_=xs_d[:, :])
            if not gamma_one:
                gam_sb = cpool.tile([P, D], f32, tag="gam")
                nc.gpsimd.dma_start(out=gam_sb[:], in_=gam_d[:, :])

            ident = cpool.tile([P, P], f32, tag="ident")
            make_identity(nc, ident[:])

            # Quake rsqrt integer constants (per-partition scalars)
            c_shift = cpool.tile([P, 1], u32, tag="cshift")
            nc.vector.memset(c_shift[:], 1)
            c_magic = cpool.tile([P, 1], u32, tag="cmagic")
            nc.vector.memset(c_magic[:], 0x5F3759DF)
            c_m05 = cpool.tile([P, 1], f32, tag="cm05")
            nc.vector.memset(c_m05[:], -0.5)
            c_15 = cpool.tile([P, 1], f32, tag="c15")
            nc.vector.memset(c_15[:], 1.5)

            # ---- group structure over this core's nodes; small tail groups
            # shrink the serial pipeline-drain at the end
            groups = []
            n0 = 0
            tails = [256, 256, 256] if NPC >= 2 * GROUP else []
            tail = sum(tails)
            while n0 < NPC - tail:
                gw = min(GROUP, NPC - tail - n0)
                groups.append((n0, gw))
                n0 += gw
            for gw in tails:
                groups.append((n0, gw))
                n0 += gw

            state = {"blk": 0, "cur_sup": -1, "oh_t": None}

            def do_agg(n0, gw):
                # ---- phase A: aggregate this group's node tiles
                agg_g = aggpool.tile([P, GROUP], f16, tag="agg")
                ntile = gw // TW
                a = 0
                while a < ntile:
                    atc = min(cfg.at, ntile - a)
                    ps = psA.tile([P, cfg.at * TW], f32, tag="psA")
                    for a2 in range(atc):
                        k = n0 // TW + a + a2
                        nblk = schedule[k]
                        for i in range(nblk):
                            blk = state["blk"]
                            bi = blk // EB
                            if bi not in ebatches:
                                fetch_batch(bi)
                            fetch_batch(bi + 1)
                            ebatches.pop(bi - 2, None)
                            sup = blk // S_OH
                            so = blk - sup * S_OH
                            lookahead = (blk + 8 * S_OH) // S_OH
                            if lookahead >= SPLIT:
                                fetch_ohd((lookahead - SPLIT) // OBS)
                            if sup >= SPLIT:
                                # DMA'd onehot super (fp8)
                                di = sup - SPLIT
                                obi = di // OBS
                                if obi not in obatches:
                                    fetch_ohd(obi)
                                fetch_ohd(obi + 1)
                                obatches.pop(obi - 2, None)
                                c0 = ((di - obi * OBS) * S_OH + so) * TW
                                rhs = obatches[obi][:, c0 : c0 + TW]
                            else:
                                if sup != state["cur_sup"]:
                                    # onehot[p, s*TW+n] = (j[.] == n);
                                    # contiguous per block for matmul rhs
                                    nbk = min(S_OH, NB - sup * S_OH)
                                    oh_t = ohpool.tile(
                                        [P, TW * S_OH], f16, tag="oh"
                                    )
                                    nc.vector.tensor_tensor(
                                        out=oh_t[:, : nbk * TW].rearrange(
                                            "p (s n) -> p s n", n=TW
                                        ),
                                        in0=wc_sb[
                                            :, WC_IOTAR : WC_IOTAR + nbk * TW
                                        ].rearrange("p (s n) -> p s n", n=TW),
                                        in1=jt_sb[
                                            :, sup * S_OH : sup * S_OH + nbk
                                        ]
                                        .unsqueeze(2)
                                        .broadcast_to([P, nbk, TW]),
                                        op=Op.is_equal,
                                    )
                                    state["cur_sup"] = sup
                                    state["oh_t"] = oh_t
                                rhs = state["oh_t"][:, so * TW
                                                    : (so + 1) * TW]
                            nc.tensor.matmul(
                                out=ps[:, a2 * TW : (a2 + 1) * TW],
                                lhsT=ebatches[bi][
                                    :, (blk - bi * EB) * D
                                    : (blk - bi * EB + 1) * D
                                ],
                                rhs=rhs,
                                start=(i == 0),
                                stop=(i == nblk - 1),
                            )
                            state["blk"] = blk + 1
                    nc.scalar.copy(
                        agg_g[:, a * TW : (a + atc) * TW], ps[:, : atc * TW]
                    )
                    a += atc
                return {"n0": n0, "gw": gw, "agg_g": agg_g}

            def stage1(st):
                # ---- MLP for this group (through h2 PSUM)
                n0, gw, agg_g = st["n0"], st["gw"], st["agg_g"]
                nsl = slice(n0, n0 + gw)
                h1a_ps = psB.tile([P, GROUP], f32, tag="h1a")
                h1b_ps = psB.tile([P, GROUP], f32, tag="h1b")
                nc.tensor.matmul(
                    out=h1a_ps[:, :gw], lhsT=wc_sb[:, WC_W1A : WC_W1A + P],
                    rhs=xt_sb[:, nsl], start=True, stop=False,
                )
                nc.tensor.matmul(
                    out=h1a_ps[:, :gw], lhsT=wc_sb[:, WC_W1B : WC_W1B + P],
                    rhs=agg_g[:, :gw], start=False, stop=True,
                )
                nc.tensor.matmul(
                    out=h1b_ps[:, :gw], lhsT=wc_sb[:, WC_W1A + P : WC_W1A + 2 * P],
                    rhs=xt_sb[:, nsl], start=True, stop=False,
                )
                nc.tensor.matmul(
                    out=h1b_ps[:, :gw], lhsT=wc_sb[:, WC_W1B + P : WC_W1B + 2 * P],
                    rhs=agg_g[:, :gw], start=False, stop=True,
                )
                h1a_sb = h1pool.tile([P, GROUP], f16, tag="h1as")
                h1b_sb = h1pool.tile([P, GROUP], f16, tag="h1bs")
                if cfg.native_silu:
                    nc.scalar.activation(
                        h1a_sb[:, :gw], h1a_ps[:, :gw], Af.Silu,
                        bias=fc_sb[:, 0:1],
                    )
                    nc.scalar.activation(
                        h1b_sb[:, :gw], h1b_ps[:, :gw], Af.Silu,
                        bias=fc_sb[:, 1:2],
                    )
                else:
                    # silu(z) = z * sigmoid(z), z = h1 + b1 (CoreSim lacks Silu)
                    for (hps, hsb, bsl) in (
                        (h1a_ps, h1a_sb, fc_sb[:, 0:1]),
                        (h1b_ps, h1b_sb, fc_sb[:, 1:2]),
                    ):
                        zpre = h1pool.tile([P, GROUP], f16, tag="zpre")
                        nc.scalar.activation(
                            zpre[:, :gw], hps[:, :gw], Af.Identity, bias=bsl
                        )
                        zsig = h1pool.tile([P, GROUP], f16, tag="zsig")
                        nc.scalar.activation(
                            zsig[:, :gw], hps[:, :gw], Af.Sigmoid, bias=bsl
                        )
                        nc.vector.tensor_tensor(
                            out=hsb[:, :gw], in0=zpre[:, :gw], in1=zsig[:, :gw],
                            op=Op.mult,
                        )
                h2_ps = psB.tile([P, GROUP], f32, tag="h2", bufs=2)
                nc.tensor.matmul(
                    out=h2_ps[:, :gw], lhsT=wc_sb[:, WC_W2A : WC_W2A + P],
                    rhs=h1a_sb[:, :gw], start=True, stop=False,
                )
                nc.tensor.matmul(
                    out=h2_ps[:, :gw], lhsT=wc_sb[:, WC_W2B : WC_W2B + P],
                    rhs=h1b_sb[:, :gw], start=False, stop=True,
                )
                st["h2_ps"] = h2_ps

            def stage2a(st):
                gw = st["gw"]
                h2_sb = h2pool.tile([P, GROUP], f32, tag="h2s")
                nc.scalar.activation(
                    h2_sb[:, :gw], st["h2_ps"][:, :gw], Af.Identity,
                    bias=fc_sb[:, 2:3],
                )
                st["h2_sb"] = h2_sb

            def stage2b(st):
                n0, gw, h2_sb = st["n0"], st["gw"], st["h2_sb"]
                # ---- transpose to [node, feat], tile pairs interleaved
                # column-wise in PSUM so one bn_stats over the contiguous
                # [P, 256] pair-region yields per-tile stats in its even/odd
                # halves (count, mean, count*var each) - no bn_aggr needed.
                nq = gw // P
                nh = (nq + 1) // 2
                tp_all = psT.tile([P, 4 * P], f32, tag="tp")
                pairviews = []
                for h in range(nh):
                    region = tp_all[:, h * 2 * P : (h + 1) * 2 * P]
                    pv = region.rearrange("p (f two) -> p f two", two=2)
                    pairviews.append(pv)
                    for t in range(2):
                        q = min(2 * h + t, nq - 1)  # odd tail: dup last tile
                        nc.tensor.transpose(
                            pv[:, :, t], h2_sb[:, q * P : (q + 1) * P],
                            ident[:],
                        )
                stats_g = stpool.tile([P, 12], f32, tag="bst")
                for h in range(nh):
                    nc.vector.bn_stats(
                        out=stats_g[:, 6 * h : 6 * h + 6],
                        in_=tp_all[:, h * 2 * P : (h + 1) * 2 * P],
                    )

                # ---- rstd = 1/sqrt(var + eps) via Quake rsqrt + Newton
                # stats cols: [1]=mean_even(q0) [2]=128*var(q0)
                #             [4]=mean_odd(q1)  [5]=128*var(q1)
                veps = stpool.tile([P, 4], f32, tag="veps")
                nc.vector.tensor_scalar(
                    out=veps[:, :nq],
                    in0=stats_g[:, 2 : 3 * nq : 3],
                    scalar1=1.0 / P, scalar2=cfg.eps,
                    op0=Op.mult, op1=Op.add,
                )
                ish = stpool.tile([P, 4], u32, tag="ish")
                nc.vector.tensor_tensor(
                    out=ish[:, :nq], in0=veps[:, :nq].bitcast(u32),
                    in1=c_shift[:, :].to_broadcast([P, nq]),
                    op=Op.arith_shift_right,
                )
                y = stpool.tile([P, 4], f32, tag="rsy")
                nc.vector.tensor_tensor(
                    out=y[:, :nq].bitcast(u32),
                    in0=c_magic[:, :].to_broadcast([P, nq]),
                    in1=ish[:, :nq], op=Op.subtract,
                )
                for it in range(cfg.newton):
                    t_ = stpool.tile([P, 4], f32, tag=f"nt{it}")
                    nc.vector.tensor_tensor(
                        out=t_[:, :nq], in0=y[:, :nq], in1=y[:, :nq], op=Op.mult
                    )
                    nc.vector.tensor_tensor(
                        out=t_[:, :nq], in0=t_[:, :nq], in1=veps[:, :nq], op=Op.mult
                    )
                    nc.vector.tensor_scalar(
                        out=t_[:, :nq], in0=t_[:, :nq], scalar1=c_m05[:, :],
                        scalar2=c_15[:, :], op0=Op.mult, op1=Op.add,
                    )
                    y2 = stpool.tile([P, 4], f32, tag=f"ny{it}")
                    nc.vector.tensor_tensor(
                        out=y2[:, :nq], in0=y[:, :nq], in1=t_[:, :nq], op=Op.mult
                    )
                    y = y2

                # ---- normalize on ScalarE: (tp - mu)*rstd = tp*rstd + (-mu*rstd)
                nmu = stpool.tile([P, 4], f32, tag="nmu")
                nc.vector.scalar_tensor_tensor(
                    out=nmu[:, :nq], in0=stats_g[:, 1 : 3 * nq : 3],
                    scalar=-1.0, in1=y[:, :nq],
                    op0=Op.mult, op1=Op.mult,
                )
                ost = ospool.tile([P, GROUP], f16, tag="ost")
                for q in range(nq):
                    k = n0 // P + q
                    t1 = lnpool.tile([P, P], f16, tag="t1")
                    if q % 2 == 0:
                        nc.scalar.activation(
                            t1[:], pairviews[q // 2][:, :, q % 2],
                            Af.Identity,
                            bias=nmu[:, q : q + 1], scale=y[:, q : q + 1],
                        )
                    else:
                        nc.vector.tensor_scalar(
                            out=t1[:], in0=pairviews[q // 2][:, :, q % 2],
                            scalar1=stats_g[:, 1 + 3 * q : 2 + 3 * q],
                            scalar2=y[:, q : q + 1],
                            op0=Op.subtract, op1=Op.mult,
                        )
                    if gamma_one:
                        t2 = t1
                    else:
                        t2 = lnpool.tile([P, P], f16, tag="t2")
                        nc.vector.tensor_tensor(
                            out=t2[:], in0=t1[:], in1=gam_sb[:], op=Op.mult
                        )
                    nc.vector.tensor_tensor(
                        out=ost[:, q * P : (q + 1) * P], in0=t2[:],
                        in1=xs_sb[:, k * D : (k + 1) * D], op=Op.add,
                    )
                nc.gpsimd.dma_start(
                    out=out_d[:, n0 : n0 + gw], in_=ost[:, :gw]
                )

            # ---- software-pipelined driver: phase-B lags aggregation by
            # one group (MLP) / two groups (LN), so no engine's in-order
            # queue head-of-line blocks on a cross-engine dependency.
            pipe = []
            for (n0, gw) in groups:
                if len(pipe) == 2:
                    stage2a(pipe[0])
                if len(pipe) >= 1:
                    stage1(pipe[-1])
                if len(pipe) == 2:
                    stage2b(pipe.pop(0))
                pipe.append(do_agg(n0, gw))
            if len(pipe) == 2:
                stage2a(pipe[0])
            stage1(pipe[-1])
            if len(pipe) == 2:
                stage2b(pipe.pop(0))
            stage2a(pipe[0])
            stage2b(pipe.pop(0))
    nc.finalize()
    return nc


# ---------------------------------------------------------------- run

def _run(inputs, cfg, use_sim=False):
    global LAST_RESULTS
    x = np.asarray(inputs["x"], dtype=np.float32)
    W1 = np.asarray(inputs["W1"], dtype=np.float16)
    W2 = np.asarray(inputs["W2"], dtype=np.float16)
    b1 = np.asarray(inputs["b1"], dtype=np.float32)
    b2 = np.asarray(inputs["b2"], dtype=np.float32)
    beta = np.asarray(inputs["ln_beta"], dtype=np.float32)
    gamma = np.asarray(inputs["ln_gamma"], dtype=np.float32)
    gamma_one = bool(np.all(gamma == 1.0))

    schedule, shards, node_ids = _prepare(
        x, inputs["edge_index"], inputs["edge_attr"], beta, cfg
    )
    nc = _build(cfg, schedule, gamma_one)

    wc = np.zeros((P, WC_COLS), dtype=np.float16)
    wc[:, WC_W1A : WC_W1A + 2 * D] = W1[0:P, :]
    wc[:, WC_W1B : WC_W1B + 2 * D] = W1[P : 2 * P, :]
    wc[:, WC_W2A : WC_W2A + D] = W2[0:P, :]
    wc[:, WC_W2B : WC_W2B + D] = W2[P : 2 * P, :]
    iota_rep = np.tile(np.arange(TW, dtype=np.float16), S_OH)
    wc[:, WC_IOTAR : WC_IOTAR + TW * S_OH] = iota_rep[None, :]
    fc = np.zeros((P, 4), dtype=np.float32)
    fc[:, 0] = b1[0:P]
    fc[:, 1] = b1[P : 2 * P]
    fc[:, 2] = b2[0:P]
    gam = np.tile(gamma[None, :], (P, 1)).astype(np.float32)

    common = {"fc": fc, "gam": gam}
    in_maps = []
    for sh in shards:
        m = dict(sh, **common)
        m["wc"] = np.concatenate([wc, m.pop("jt")], axis=1)
        in_maps.append(m)

    if use_sim:
        from concourse import bass_interp

        outs = []
        for c in range(N_CORES):
            sim = bass_interp.MultiCoreSim(nc, 1)
            for name, arr in in_maps[c].items():
                sim.cores[0].tensor(name)[:] = arr
            sim.simulate()
            outs.append(np.array(sim.cores[0].mem_tensor("out")))
    else:
        from concourse.bass_utils import run_bass_kernel_spmd

        res = run_bass_kernel_spmd(nc, in_maps, list(range(N_CORES)))
        LAST_RESULTS = res
        outs = [res.results[c]["out"] for c in range(N_CORES)]

    out_full = np.zeros((cfg.nta * TW, D), dtype=np.float32)
    for c in range(N_CORES):
        out_pm = np.asarray(outs[c])
        out_rows = (
            out_pm.reshape(P, cfg.tpc, D).transpose(1, 0, 2).reshape(cfg.npc, D)
        )
        out_full[node_ids[c]] = out_rows.astype(np.float32)
    return out_full[: cfg.n_nodes]


def kernel(**inputs):
    return _run(inputs, REAL_CFG, use_sim=False)
